# revision 1
# baseline (speedup 1.0000x reference)
"""AWQ W4A16 linear kernel for Trainium2 (8 NeuronCores, tensor-parallel).

y = x @ dequant(qweight, wscales, wzeros)^T + bias
  x:       [4096, 4096] fp32
  qweight: [12288, 2048] int32 (2 uint4 per value, low nibble = even k)
  wscales: [32, 12288] fp32   (per group of 128 k)
  wzeros:  [32, 12288] fp32
  bias:    [12288] fp32
  out:     [4096, 12288] fp32

Strategy: column-parallel across 8 cores (each core owns 1536 output
columns). The weights are dequantized host-side to fp16 in [K, N] layout;
x is transposed host-side to [K, M] fp16. Each core runs a dense GEMM:
out tiles y[m:128, :1536] accumulated over 32 k-tiles in PSUM, bias added
on the DVE during PSUM->SBUF eviction.
"""

import numpy as np

import concourse.bass as bass
import concourse.mybir as mybir
import concourse.tile as tile
from concourse import bacc
from concourse.bass_utils import run_bass_kernel_spmd

M, K, N = 4096, 4096, 12288
GROUP = 128
NCORES = 8
NS = N // NCORES          # 1536 out columns per core
P = 128
KT = K // P               # 32 k tiles
NCHUNK = 512
NCH = NS // NCHUNK        # 3 psum banks per m tile
MBLK = 512                # m block held in SBUF at once
MB = M // MBLK            # 8
MSUB = MBLK // P          # 4

_DT = mybir.dt.float16
_NP_DT = np.float16


def _build():
    nc = bacc.Bacc(None, target_bir_lowering=False)
    xt = nc.dram_tensor("xt", [K, M], _DT, kind="ExternalInput")
    wt = nc.dram_tensor("wt", [K, NS], _DT, kind="ExternalInput")
    bb = nc.dram_tensor("bb", [P, NS], mybir.dt.float32, kind="ExternalInput")
    y = nc.dram_tensor("y", [M, NS], mybir.dt.float32, kind="ExternalOutput")

    with tile.TileContext(nc) as tc:
        with (
            tc.tile_pool(name="wpool", bufs=1) as wpool,
            tc.tile_pool(name="bpool", bufs=1) as bpool,
            tc.tile_pool(name="xpool", bufs=2) as xpool,
            tc.tile_pool(name="opool", bufs=2) as opool,
            tc.tile_pool(name="psum", bufs=2, space="PSUM") as psum,
        ):
            w_sb = wpool.tile([P, KT, NS], _DT)
            wt_r = wt.rearrange("(kt p) n -> p kt n", p=P)
            for kt in range(KT):
                nc.sync.dma_start(w_sb[:, kt, :], wt_r[:, kt, :])
            bias_sb = bpool.tile([P, NS], mybir.dt.float32)
            nc.sync.dma_start(bias_sb[:], bb[:, :])

            xt_r = xt.rearrange("(kt p) m -> p kt m", p=P)
            for mb in range(MB):
                x_sb = xpool.tile([P, KT, MBLK], _DT)
                nc.sync.dma_start(
                    x_sb[:], xt_r[:, :, mb * MBLK:(mb + 1) * MBLK]
                )
                for ms in range(MSUB):
                    psts = [
                        psum.tile([P, NCHUNK], mybir.dt.float32, name=f"ps{i}")
                        for i in range(NCH)
                    ]
                    lhs = x_sb[:, :, ms * P:(ms + 1) * P]
                    for kt in range(KT):
                        for i in range(NCH):
                            nc.tensor.matmul(
                                psts[i][:],
                                lhs[:, kt, :],
                                w_sb[:, kt, i * NCHUNK:(i + 1) * NCHUNK],
                                start=(kt == 0),
                                stop=(kt == KT - 1),
                            )
                    out_sb = opool.tile([P, NS], mybir.dt.float32)
                    for i in range(NCH):
                        nc.vector.tensor_add(
                            out_sb[:, i * NCHUNK:(i + 1) * NCHUNK],
                            psts[i][:],
                            bias_sb[:, i * NCHUNK:(i + 1) * NCHUNK],
                        )
                    m0 = mb * MBLK + ms * P
                    nc.sync.dma_start(y[m0:m0 + P, :], out_sb[:, :])
    nc.compile()
    return nc


_nc_cache = None


def _get_nc():
    global _nc_cache
    if _nc_cache is None:
        _nc_cache = _build()
    return _nc_cache


def _dequant_wt(qweight, wscales, wzeros):
    """Return w^T [K, N] fp16: w[n,k] = (wint[n,k] - z[g,n]) * s[g,n]."""
    qw = qweight.astype(np.int32)
    low = (qw & 0xF).astype(np.float32)          # [N, K//2] -> even k
    high = ((qw >> 4) & 0xF).astype(np.float32)  # odd k
    G = K // GROUP
    # Build [K, N] directly: interleave along k axis.
    wiT = np.empty((K, qw.shape[0]), dtype=np.float32)
    wiT[0::2, :] = low.T
    wiT[1::2, :] = high.T
    wg = wiT.reshape(G, GROUP, -1)
    wg -= wzeros[:, None, :]
    wg *= wscales[:, None, :]
    return wg.reshape(K, -1).astype(_NP_DT)


def prepare_inputs(x, qweight, wscales, wzeros, bias):
    xt16 = x.T.astype(_NP_DT)  # [K, M], C-contiguous after astype
    wt16 = _dequant_wt(qweight, wscales, wzeros)  # [K, N]
    in_maps = []
    for c in range(NCORES):
        sl = slice(c * NS, (c + 1) * NS)
        in_maps.append({
            "xt": xt16,
            "wt": np.ascontiguousarray(wt16[:, sl]),
            "bb": np.ascontiguousarray(
                np.broadcast_to(bias[sl].astype(np.float32), (P, NS))
            ),
        })
    return in_maps


def kernel(x, qweight, wscales, wzeros, bias):
    nc = _get_nc()
    in_maps = prepare_inputs(x, qweight, wscales, wzeros, bias)
    res = run_bass_kernel_spmd(nc, in_maps, core_ids=list(range(NCORES)))
    return np.concatenate(
        [res.results[c]["y"] for c in range(NCORES)], axis=1
    ).astype(np.float32)


# revision 2
# speedup vs baseline: 56.5063x; 56.5063x over previous
"""AWQ W4A16 linear kernel for Trainium2 (8 NeuronCores, tensor-parallel).

y = x @ dequant(qweight, wscales, wzeros)^T + bias
  x:       [4096, 4096] fp32
  qweight: [12288, 2048] int32 (2 uint4 per value, low nibble = even k)
  wscales: [32, 12288] fp32   (per group of 128 k)
  wzeros:  [32, 12288] fp32
  bias:    [12288] fp32
  out:     [4096, 12288] fp32

Strategy: column-parallel across 8 cores (each core owns 1536 output
columns). The weights are dequantized host-side to fp16 in [K, N] layout;
x is transposed host-side to [K, M] fp16. Each core runs a dense GEMM:
out tiles y[m:128, :1536] accumulated over 32 k-tiles in PSUM, bias added
on the DVE during PSUM->SBUF eviction.
"""

import numpy as np

import concourse.bass as bass
import concourse.mybir as mybir
import concourse.tile as tile
from concourse import bacc
from concourse.bass_utils import run_bass_kernel_spmd

M, K, N = 4096, 4096, 12288
GROUP = 128
NCORES = 8
NS = N // NCORES          # 1536 out columns per core
P = 128
KT = K // P               # 32 k tiles
NCHUNK = 512
NCH = NS // NCHUNK        # 3 psum banks per m tile
MBLK = 512                # m block held in SBUF at once
MB = M // MBLK            # 8
MSUB = MBLK // P          # 4

_DT = mybir.dt.float16
_NP_DT = np.float16


def _build(repeat=1):
    from contextlib import nullcontext

    nc = bacc.Bacc(None, target_bir_lowering=False)
    xt = nc.dram_tensor("xt", [K, M], _DT, kind="ExternalInput")
    wt = nc.dram_tensor("wt", [K, NS], _DT, kind="ExternalInput")
    bb = nc.dram_tensor("bb", [P, NS], mybir.dt.float32, kind="ExternalInput")
    y = nc.dram_tensor("y", [M, NS], mybir.dt.float32, kind="ExternalOutput")

    with tile.TileContext(nc) as tc:
        with (
            tc.tile_pool(name="wpool", bufs=1) as wpool,
            tc.tile_pool(name="bpool", bufs=1) as bpool,
            tc.tile_pool(name="xpool", bufs=2) as xpool,
            tc.tile_pool(name="opool", bufs=2) as opool,
            tc.tile_pool(name="psum", bufs=2, space="PSUM") as psum,
        ):
            w_sb = wpool.tile([P, KT, NS], _DT)
            wt_r = wt.rearrange("(kt p) n -> p kt n", p=P)
            for kt in range(KT):
                nc.sync.dma_start(w_sb[:, kt, :], wt_r[:, kt, :])
            bias_sb = bpool.tile([P, NS], mybir.dt.float32)
            nc.sync.dma_start(bias_sb[:], bb[:, :])

            xt_r = xt.rearrange("(kt p) m -> p kt m", p=P)
            loop = tc.For_i(0, repeat, 1) if repeat != 1 else nullcontext()
            with loop:
                for mb in range(MB):
                    x_sb = xpool.tile([P, KT, MBLK], _DT)
                    nc.sync.dma_start(
                        x_sb[:], xt_r[:, :, mb * MBLK:(mb + 1) * MBLK]
                    )
                    for ms in range(MSUB):
                        psts = [
                            psum.tile([P, NCHUNK], mybir.dt.float32,
                                      name=f"ps{i}")
                            for i in range(NCH)
                        ]
                        lhs = x_sb[:, :, ms * P:(ms + 1) * P]
                        for kt in range(KT):
                            for i in range(NCH):
                                nc.tensor.matmul(
                                    psts[i][:],
                                    lhs[:, kt, :],
                                    w_sb[:, kt, i * NCHUNK:(i + 1) * NCHUNK],
                                    start=(kt == 0),
                                    stop=(kt == KT - 1),
                                )
                        out_sb = opool.tile([P, NS], mybir.dt.float32)
                        for i in range(NCH):
                            nc.vector.tensor_add(
                                out_sb[:, i * NCHUNK:(i + 1) * NCHUNK],
                                psts[i][:],
                                bias_sb[:, i * NCHUNK:(i + 1) * NCHUNK],
                            )
                        m0 = mb * MBLK + ms * P
                        nc.sync.dma_start(y[m0:m0 + P, :], out_sb[:, :])
    nc.compile()
    return nc


_nc_cache = None


def _get_nc():
    global _nc_cache
    if _nc_cache is None:
        _nc_cache = _build()
    return _nc_cache


def _dequant_wt(qweight, wscales, wzeros):
    """Return w^T [K, N] fp16: w[n,k] = (wint[n,k] - z[g,n]) * s[g,n]."""
    qw = qweight.astype(np.int32)
    low = (qw & 0xF).astype(np.float32)          # [N, K//2] -> even k
    high = ((qw >> 4) & 0xF).astype(np.float32)  # odd k
    G = K // GROUP
    # Build [K, N] directly: interleave along k axis.
    wiT = np.empty((K, qw.shape[0]), dtype=np.float32)
    wiT[0::2, :] = low.T
    wiT[1::2, :] = high.T
    wg = wiT.reshape(G, GROUP, -1)
    wg -= wzeros[:, None, :]
    wg *= wscales[:, None, :]
    return wg.reshape(K, -1).astype(_NP_DT)


def prepare_inputs(x, qweight, wscales, wzeros, bias):
    xt16 = x.T.astype(_NP_DT)  # [K, M], C-contiguous after astype
    wt16 = _dequant_wt(qweight, wscales, wzeros)  # [K, N]
    in_maps = []
    for c in range(NCORES):
        sl = slice(c * NS, (c + 1) * NS)
        in_maps.append({
            "xt": xt16,
            "wt": np.ascontiguousarray(wt16[:, sl]),
            "bb": np.ascontiguousarray(
                np.broadcast_to(bias[sl].astype(np.float32), (P, NS))
            ),
        })
    return in_maps


def kernel(x, qweight, wscales, wzeros, bias):
    nc = _get_nc()
    in_maps = prepare_inputs(x, qweight, wscales, wzeros, bias)
    res = run_bass_kernel_spmd(nc, in_maps, core_ids=list(range(NCORES)))
    return np.concatenate(
        [res.results[c]["y"] for c in range(NCORES)], axis=1
    ).astype(np.float32)
